# revision 7
# baseline (speedup 1.0000x reference)
"""Depthwise cross-correlation (SiamFC-style) Trainium2 kernel.

z: [128, 256, 7, 7] templates, x: [128, 256, 31, 31] search images.
out[b,c,p,q] = sum_{i,j} z[b,c,i,j] * x[b,c,p+i,q+j]  -> [128, 256, 25, 25]

Strategy: pure data parallel over batch (16 batches/core on 8 cores).
Per core: 4096 (b,c) channel pairs = 32 tiles of 128 partitions.

Per tile the 49 filter taps are split across all four compute engines:
 - N_PE taps as diagonal-weight bf16 matmuls accumulating in PSUM (two
   chunks: p-rows [0,13) and [13,25), one bank each).  The diagonal
   weights are prebuilt on the host and DMAed per tile, so no on-chip
   weight builds are needed.
 - N_ACT taps as ScalarE per-partition-scale products into bf16 slots.
 - N_DVE taps as VectorE tensor_scalar products (4x perf mode).
 - N_POOL taps as GpSimd fused scalar_tensor_tensor multiply-accumulate
   into a private bf16 accumulator (no separate add pass).
ACT/DVE products are summed by a grouped binary tree of bf16 VectorE
adds (2x mode) together with the GpSimd accumulator, then merged with
the PSUM chunks into the f32 output tile.

bf16 inputs/products keep the datapath in the DVE fast modes and halve
DMA; PSUM accumulation stays f32.  x rows are padded to pitch 32 in DRAM
so the input DMA is one contiguous run per partition.
"""

import numpy as np

B, C = 128, 256
HZ, WZ = 7, 7
HX, WX = 31, 31
HO, WO = 25, 25
N_CORES = 8
B_PER_CORE = B // N_CORES            # 16
PAIRS = B_PER_CORE * C               # 4096 channel pairs per core
NTILES = PAIRS // 128                # 32
WXP = 32                             # padded x row pitch (DRAM and SBUF)
ZF = HZ * WZ                         # 49
OF = HO * WO                         # 625
WQ = 25                              # q-window (odd innermost is fine for bf16 matmul)
P_SPLIT = 13                         # PSUM chunk A rows; B gets HO-P_SPLIT

# tap assignment: first N_PE taps -> PE, then N_ACT -> ScalarE products,
# then N_DVE -> VectorE products, last N_POOL -> GpSimd products.
# N_FOLD slots are folded by one accumulating SBUF->SBUF DMA (GpSimd
# SWDGE) before the VectorE binary add tree; the final slot sum S is
# absorbed into PSUM by two identity matmuls, and the output is DMAed
# straight from PSUM (no eviction op, no final merge).
N_PE = 26
N_ACT = 9
N_DVE = 8
N_POOL = ZF - N_PE - N_ACT - N_DVE   # 6
N_DMA_MERGE = 7
# PE warmup: dummy matmuls issued during the initial DMA fill so the
# tensor engine leaves its cold p-state before real work arrives
N_WARMUP = 8


def _split_for(t, ntiles):
    """Per-tile engine split (n_pe, n_act, n_dve, n_pool, add_mode).
    Steady tiles use the balanced split with the grouped add tree."""
    if t == 0:
        return (N_PE, 6, 11, 6, "tree")
    return (N_PE, N_ACT, N_DVE, N_POOL, "tree")


def _max_pe(ntiles=NTILES):
    return max(_split_for(t, ntiles)[0] for t in range(ntiles))


def _install_tilefix():
    """This walrus build accepts only one sync-wait command on a Drain.
    Split the TileContext tail-drain waits across single-wait SP nops."""
    import concourse.tile as tile_mod
    from concourse.vector_clock import ScopedClock

    def _drain_and_barrier_split(self, tick_clock, wait_clock):
        nc = self.nc
        probe = nc.sync.nop(nofuse=True, hint="drain_wait_probe")
        wait_clock.add_sem_waits(
            probe.ins, ScopedClock({None: tick_clock.global_clock})
        )
        si = probe.ins.sync_info
        waits = list(si.on_wait) if si is not None and si.on_wait else []
        if si is not None:
            si.on_wait = waits[:1]
        for w in waits[1:]:
            stub = nc.sync.nop(nofuse=True, hint="drain_wait_split")
            ssi = stub.ins.sync_info
            if ssi is None:
                import concourse.mybir as mybir
                stub.ins.sync_info = mybir.SyncInfo(on_wait=[w], on_update=[])
            else:
                ssi.on_wait = list(ssi.on_wait or []) + [w]
        nc.sync.drain()
        nc.all_engine_barrier()
        assert self.sems is not None
        popped = nc._tile_sem_poison_stack.pop()
        assert popped is self._sem_poison
        nc.clear_and_free_semaphores(list(self.sems.allocated().values()))
        nc.all_engine_barrier()

    tile_mod.TileContext._drain_and_barrier = _drain_and_barrier_split


def _split_multi_waits(nc):
    """This walrus build accepts only one sync-wait command per instruction.
    Hoist extra waits onto single-wait nops on the same engine just before."""
    import concourse.mybir as mybir

    n = 0
    for f in nc.m.functions:
        for bb in f.blocks:
            insts = list(bb.instructions)
            out_insts = []
            changed = False
            for inst in insts:
                si = inst.sync_info
                if si is not None and si.on_wait and len(si.on_wait) > 1:
                    waits = list(si.on_wait)
                    si.on_wait = waits[-1:]
                    for w in waits[:-1]:
                        n += 1
                        out_insts.append(mybir.InstNoOp(
                            name=f"waitsplit-{n}",
                            engine=inst.engine,
                            bass_nofuse=True,
                            sync_info=mybir.SyncInfo(on_wait=[w], on_update=[]),
                        ))
                    changed = True
                out_insts.append(inst)
            if changed:
                bb.instructions.clear()
                for inst in out_insts:
                    bb.add_instruction(inst)
    return n


_NC_CACHE = {}


def _build_bass(reps: int = 1, timing: bool = False, ntiles: int = NTILES):
    import concourse.bass as bass
    import concourse.mybir as mybir
    import concourse.tile as tile

    _install_tilefix()

    f32 = mybir.dt.float32
    bf16 = mybir.dt.bfloat16
    nc = bass.Bass()
    xs = nc.declare_dram_parameter("xs", [PAIRS, HX, WXP], bf16, isOutput=False)
    zs = nc.declare_dram_parameter("zs", [PAIRS, ZF], f32, isOutput=False)
    n_pe_ws = _max_pe(ntiles)
    ws = nc.declare_dram_parameter("ws", [PAIRS, n_pe_ws, 128], bf16,
                                   isOutput=False)
    out_rows = 128 if timing else PAIRS
    out = nc.declare_dram_parameter("out", [out_rows, OF], bf16, isOutput=True)

    N_BUILD0 = 3
    from concourse.masks import make_identity

    with tile.TileContext(nc) as tc:
        with (
            tc.tile_pool(name="consts", bufs=1) as consts,
            tc.tile_pool(name="xin", bufs=3) as xin,
            tc.tile_pool(name="zin", bufs=3) as zin,
            tc.tile_pool(name="win", bufs=3) as win_pool,
            tc.tile_pool(name="prodp", bufs=3) as prodp,
            tc.tile_pool(name="outp", bufs=4) as outp,
            tc.tile_pool(name="psum", bufs=3, space="PSUM") as psum,
            tc.tile_pool(name="wups", bufs=1, space="PSUM") as wups,
        ):
            def win(x_t, i, j, p0, pc):
                return x_t[:, i + p0:i + p0 + pc, j:j + WQ]

            ident_f = consts.tile([128, 128], f32)
            make_identity(nc, ident_f)
            ident = consts.tile([128, 128], bf16)
            nc.vector.tensor_copy(ident, ident_f)
            w0 = consts.tile([128, N_BUILD0, 128], bf16)

            if N_WARMUP:
                # spin the tensor engine during the initial DMA fill so the
                # p-state ramp completes before the first real matmul
                wu_w = xin.tile([128, 128], bf16)
                wu_x = xin.tile([128, 512], bf16)
                wu_ps = wups.tile([128, 512], f32)
                nc.vector.memset(wu_w, 0.0)
                nc.vector.memset(wu_x, 0.0)
                for _ in range(N_WARMUP):
                    nc.tensor.matmul(wu_ps, wu_w, wu_x, start=True,
                                     stop=True, skip_group_check=True)

            def flush_out():
                while pending_out:
                    po0, po_t = pending_out.pop(0)
                    nc.sync.dma_start(
                        out=out[po0:po0 + 128, :],
                        in_=po_t.rearrange("p h w -> p (h w)"))

            pending_out = []
            for _rep in range(reps):
              for t in range(ntiles):
                n_pe, n_act, n_dve, n_pool, add_mode = _split_for(t, ntiles)
                r0 = t * 128
                x_t = xin.tile([128, HX, WXP], bf16)
                nc.sync.dma_start(out=x_t, in_=xs[r0:r0 + 128])
                z_t = zin.tile([128, ZF], f32)
                nc.sync.dma_start(out=z_t, in_=zs[r0:r0 + 128, :])
                w_t = win_pool.tile([128, n_pe, 128], bf16)
                nc.sync.dma_start(out=w_t, in_=ws[r0:r0 + 128, 0:n_pe])
                flush_out()

                ps_a = psum.tile([128, P_SPLIT, WQ], f32)
                ps_b = psum.tile([128, HO - P_SPLIT, WQ], f32)

                # PE route: prebuilt diag-weight bf16 matmuls into PSUM.
                # Tile 0 builds its first taps on-chip so PE starts before
                # the weights DMA completes (DMA itself is unchanged).
                first = (_rep == 0 and t == 0)
                if first:
                    for tap in range(N_BUILD0):
                        nc.vector.tensor_scalar_mul(
                            w0[:, tap, :], ident, z_t[:, tap:tap + 1])
                m_off_pre = n_act + n_dve + n_pool
                for tap in range(n_pe):
                    i, j = divmod(tap, WZ)
                    wsrc = (w0[:, tap, :] if first and tap < N_BUILD0
                            else w_t[:, tap, :])
                    last = (tap == n_pe - 1 and m_off_pre == 0)
                    nc.tensor.matmul(
                        ps_a, wsrc, win(x_t, i, j, 0, P_SPLIT),
                        start=(tap == 0), stop=last,
                        skip_group_check=True,
                    )
                    nc.tensor.matmul(
                        ps_b, wsrc,
                        win(x_t, i, j, P_SPLIT, HO - P_SPLIT),
                        start=(tap == 0), stop=last,
                        skip_group_check=True,
                    )

                # off-route products into bf16 slots (ScalarE / VectorE /
                # GpSimd tensor_scalar; the 3-input fused op is not legal on
                # GpSimd in HW).  Slot order interleaves engines so early
                # slots complete early for the add tree.
                def win25(x_t, i, j):
                    return x_t[:, i:i + HO, j:j + WO]

                m_off = n_act + n_dve + n_pool
                # slot pitch 626 keeps every slot 4B-aligned so the real
                # DVE 2x packed mode stays available for the add tree
                prods = prodp.tile([128, max(m_off, 1), OF + 1], bf16)
                for k in range(m_off):
                    tap = n_pe + k
                    i, j = divmod(tap, WZ)
                    x_w = win25(x_t, i, j)
                    zcol = z_t[:, tap:tap + 1]
                    dst = prods[:, k, 0:OF]
                    if k < n_act:
                        nc.scalar.mul(dst, x_w, zcol)
                    elif k < n_act + n_pool:
                        nc.gpsimd.tensor_scalar_mul(dst, x_w, zcol)
                    else:
                        nc.vector.tensor_scalar_mul(dst, x_w, zcol)

                # fold the top N_DMA_MERGE slots into the bottom ones
                # with one accumulating DMA (GpSimd SWDGE), then run
                # the grouped binary-tree adds on DVE (bf16 2x mode)
                live = m_off
                k = N_DMA_MERGE
                if k and m_off >= 2 * k:
                    nc.gpsimd.dma_start(
                        out=prods[:, 0:k, :],
                        in_=prods[:, m_off - k:m_off, :],
                        accum_op=mybir.AluOpType.add)
                    live = m_off - k
                while live > 1:
                    half = live // 2
                    nc.vector.tensor_add(
                        prods[:, 0:half, :],
                        prods[:, 0:half, :],
                        prods[:, half:2 * half, :])
                    if live % 2:
                        nc.vector.tensor_add(
                            prods[:, 0:1, :],
                            prods[:, 0:1, :],
                            prods[:, 2 * half:2 * half + 1, :])
                    live = half

                # absorb the slot sum S into PSUM with two identity
                # matmuls (ends the accumulation groups); evict PSUM to
                # bf16 SBUF (chunk A on ScalarE, chunk B on VectorE) and
                # DMA the bf16 tile out
                if m_off:
                    S = prods[:, 0, 0:OF].rearrange("p (h w) -> p h w", h=HO)
                    nc.tensor.matmul(
                        ps_a, ident, S[:, 0:P_SPLIT, :],
                        start=False, stop=True, skip_group_check=True)
                    nc.tensor.matmul(
                        ps_b, ident, S[:, P_SPLIT:HO, :],
                        start=False, stop=True, skip_group_check=True)
                o_t = outp.tile([128, HO, WO], bf16)
                nc.scalar.copy(o_t[:, 0:P_SPLIT, :], ps_a[:, :, 0:WO])
                nc.vector.tensor_copy(o_t[:, P_SPLIT:HO, :],
                                      ps_b[:, :, 0:WO])
                o0 = 0 if timing else r0
                pending_out.append((o0, o_t))

            flush_out()

    _split_multi_waits(nc)
    return nc


def _get_nc(reps: int = 1, timing: bool = False):
    key = ("nc", reps, timing)
    if key not in _NC_CACHE:
        _NC_CACHE[key] = _build_bass(reps, timing)
    return _NC_CACHE[key]


def _prep_inputs(z: np.ndarray, x: np.ndarray):
    import ml_dtypes

    z = np.ascontiguousarray(z, dtype=np.float32)
    x = np.ascontiguousarray(x, dtype=np.float32)
    assert z.shape == (B, C, HZ, WZ) and x.shape == (B, C, HX, WX)
    bf = ml_dtypes.bfloat16
    xp = np.zeros((B * C, HX, WXP), dtype=bf)
    xp[:, :, 0:WX] = x.reshape(B * C, HX, WX).astype(bf)
    zf = np.ascontiguousarray(z.reshape(B * C, ZF))
    # prebuilt diagonal weights for the PE taps: ws[r, k, c] =
    # (c == r % 128) ? z[r, pe_tap_k] : 0
    n_pe_ws = _max_pe()
    wsb = np.zeros((B * C, n_pe_ws, 128), dtype=bf)
    rows = np.arange(B * C)
    wsb[rows[:, None], np.arange(n_pe_ws)[None, :], (rows % 128)[:, None]] = \
        zf[:, 0:n_pe_ws].astype(bf)
    return zf, xp, wsb


def kernel(z: np.ndarray, x: np.ndarray, _trace: bool = False):
    from concourse.bass_utils import run_bass_kernel_spmd

    zf, xp, wsb = _prep_inputs(z, x)

    nc = _get_nc()
    in_maps = []
    for c in range(N_CORES):
        p0 = c * PAIRS
        in_maps.append({
            "xs": xp[p0:p0 + PAIRS],
            "zs": zf[p0:p0 + PAIRS],
            "ws": wsb[p0:p0 + PAIRS],
        })
    res = run_bass_kernel_spmd(nc, in_maps, list(range(N_CORES)), trace=_trace)
    out = np.empty((B, C, HO, WO), dtype=np.float32)
    for c in range(N_CORES):
        b0 = c * B_PER_CORE
        out[b0:b0 + B_PER_CORE] = res.results[c]["out"].reshape(
            B_PER_CORE, C, HO, WO)
    if _trace:
        return out, res
    return out

